# revision 29
# baseline (speedup 1.0000x reference)
"""LocallyConnected2d (B=8, C_in=32, 48x48, C_out=32, 3x3, pad 1) on 8 trn2 cores.

Strategy: shard the spatial-location axis L = H*W across cores (6 image rows
each). Per location l the op is an (8x288)@(288x32) GEMM with location-unique
weights; weight streaming (~5.4 MB/core fp16) dominates -> memory-bound.

Device mapping per core:
  - x halo slice lives in SBUF replicated 3x with kw column shifts, laid out
    [p=(kw*32+c), (row, col, b)], so the im2col patch for any location is a
    plain strided AP slice. Partition 96 is constant 1.0 (bias row).
  - One matmul covers FOUR consecutive locations: stationary = x-view
    [97, 32] (4 locs x 8 batch), moving = W slice [97, 128] (4 locs x 32
    out-ch), out = PSUM [32, 128].  Only the block-diagonal (loc_i == loc_j)
    quarter of the output is meaningful; garbage blocks are never read.
    This cuts the instruction count 4x vs one-matmul-per-location and makes
    every moving stream 128 wide.
  - Contraction (d=288) runs as 3 kh-rounds of K=96, PSUM-accumulated, plus
    a 97th row: stationary row 96 = 1.0, moving row 96 = bias (kh=0 round)
    or 0 -> bias is folded into the weight stream, no extra instructions.
  - 4 location-quads pack onto the PE with tile_position=(0, 32j); a 16-loc
    group = 12 matmuls -> one [128, 128] PSUM tile.
  - W is host-permuted into location-range tiles ([48 x5, 32, 16] locs,
    [96, <=9216B] rows) fully contiguous in HBM, all on the gpsimd SWDGE
    queue whose descriptors spread over the 16 DMA engines ONLY when the
    partition count is a multiple of 16 (hence 96-row weight DMAs +
    separate 1-row bias DMAs + DVE memset of the ones row).  Small tail
    tiles shrink the post-stream matmul flush.  x and bias ride the two
    HWDGE queues (hardware generation, engine 0) keeping the SWDGE ring
    pure weights.
  - Output is copied PSUM->SBUF as fp16 and streamed out in 2 DMAs on the
    gpsimd queue (the first, ready mid-stream, sits in the FIFO ring right
    behind the weights); host unscrambles to NCHW and casts to fp32.
"""

import numpy as np

import concourse.bacc as bacc
import concourse.tile as tile
from concourse import mybir
from concourse.bass_utils import run_bass_kernel_spmd

B, C_IN, H, W = 8, 32, 48, 48
C_OUT = 32
N_CORES = 8
RP = H // N_CORES  # rows per core (6)
LP = RP * W  # locations per core (288)
NGRP = LP // 16  # 16-loc output groups per core (18)

DT16 = True  # fp16 operand path (halves weight traffic)
DT = mybir.dt.float16 if DT16 else mybir.dt.float32
NPDT = np.float16 if DT16 else np.float32
F32 = mybir.dt.float32

KC = 97  # contraction rows: 96 = (kw, c), row 96 = ones/bias
XF = (RP + 2) * W * B  # x3 free size (3072)
XB = (RP + 2) * 50 * B  # xbase free size: 8 halo rows x 50 padded cols (3200)
RF = W * B  # one image row of x3 free elems (384)

LG = 48  # nominal locations per weight tile
WT_SIZES = [48, 48, 48, 48, 48, 32, 16]  # small tail tiles shrink the
WT_BASE = [0, 48, 96, 144, 192, 240, 272]  # post-stream matmul flush
NT = len(WT_SIZES)
WF = 3 * LG * C_OUT  # max weight tile free size (4608 elems = 9216B rows)

_nc = None


def _tile_of(l):
    for t in range(NT - 1, -1, -1):
        if l >= WT_BASE[t]:
            return t
    raise AssertionError


def _build():
    nc = bacc.Bacc(
        "TRN2", target_bir_lowering=False, debug=False, num_devices=N_CORES
    )
    stat = nc.dram_tensor("stat", [32, 32 + XB], DT, kind="ExternalInput")
    wds = [
        nc.dram_tensor(f"w{i}", [96, 3 * n * C_OUT], DT, kind="ExternalInput")
        for i, n in enumerate(WT_SIZES)
    ]
    bds = [
        nc.dram_tensor(f"b{i}", [1, n * C_OUT], DT, kind="ExternalInput")
        for i, n in enumerate(WT_SIZES)
    ]
    out = nc.dram_tensor("out", [128, NGRP * 128], DT, kind="ExternalOutput")

    with tile.TileContext(nc) as tc:
        with (
            tc.tile_pool(name="xpool", bufs=1) as xpool,
            tc.tile_pool(name="wpool", bufs=NT) as wpool,
            tc.tile_pool(name="opool", bufs=1) as opool,
            tc.tile_pool(name="pspool", bufs=6, space="PSUM") as pspool,
            tc.tile_pool(name="repps", bufs=2, space="PSUM") as repps,
        ):
            x3 = xpool.tile([KC, XF], DT, tag="x3")
            xb = xpool.tile([32, 32 + XB], DT, tag="xb")
            out_sb = opool.tile([128, NGRP * 128], DT)

            # all bulk DMA rides the gpsimd SWDGE queue: a dma_start whose
            # partition count is a multiple of 16 spreads its descriptors
            # evenly over all 16 DMA engines; any other count pins the whole
            # transfer to ONE engine (~13 GB/s) and poisons that engine's
            # FIFO.  So weights go as [96, 9216B] transfers and the 97th
            # (bias) row rides a separate single-descriptor DMA on SP HWDGE.
            # ones row via DVE memset keeps the Pool sequencer free to start
            # generating the big stream at once.
            nc.vector.memset(x3[96:97, 0:XF], 1.0)
            # x rides HBM->SBUF once, unreplicated: [32, 8 halo rows x 50
            # padded cols x 8 batch] plus a leading 32x32 identity.  The
            # three kw-shifted x3 partition blocks are rebuilt on-chip (DVE
            # copies for kw=0; PE identity-matmul partition shifts + DVE
            # casts for kw=1,2), saving 0.4 MB of DMA-engine critical path.
            # xb rides the SP HWDGE queue (generated in hardware, engine 0):
            # it leaves the SWDGE ring pure weights, which start ~1.3us
            # earlier; x3 replication finishes long before the stream ends.
            nc.sync.dma_start(xb[0:32, 0 : 32 + XB], stat[:, :])
            wts = []
            for t, n in enumerate(WT_SIZES):
                wt = wpool.tile([KC, WF], DT, tag="wt")
                nc.gpsimd.dma_start(wt[0:96, 0 : 3 * n * C_OUT], wds[t][:, :])
                nc.scalar.dma_start(wt[96:97, 0 : n * C_OUT], bds[t][:, :])
                wts.append(wt)

            for r in range(RP + 2):
                src0 = 32 + r * 50 * B
                nc.vector.tensor_copy(
                    x3[0:32, r * RF : (r + 1) * RF],
                    xb[0:32, src0 : src0 + RF],
                )
                for kw in (1, 2):
                    pr = repps.tile([128, 512], F32)
                    nc.tensor.matmul(
                        pr[32 * kw : 32 * kw + 32, 0:RF],
                        xb[0:32, 0:32],
                        xb[0:32, src0 + kw * B : src0 + kw * B + RF],
                        start=True,
                        stop=True,
                        skip_group_check=True,
                        tile_position=(0, 32 * kw),
                    )
                    nc.vector.tensor_copy(
                        x3[32 * kw : 32 * kw + 32, r * RF : (r + 1) * RF],
                        pr[32 * kw : 32 * kw + 32, 0:RF],
                    )

            for gi in range(NGRP):
                rl, qg = divmod(gi, 3)
                ps = pspool.tile([128, 512], F32)
                for kh in range(3):
                    kc = KC if kh == 0 else 96  # bias row rides kh=0 only
                    for j in range(4):
                        q0 = qg * 16 + 4 * j
                        l = rl * W + q0
                        t = _tile_of(l)
                        lgt = WT_SIZES[t]
                        ll = l - WT_BASE[t]
                        off = ((rl + kh) * W + q0) * B
                        nc.tensor.matmul(
                            ps[32 * j : 32 * j + 32, 0:128],
                            x3[0:kc, off : off + 32],
                            wts[t][0:kc, (kh * lgt + ll) * 32 : (kh * lgt + ll + 4) * 32],
                            start=(kh == 0),
                            stop=(kh == 2),
                            skip_group_check=True,
                            tile_position=(0, 32 * j),
                        )
                nc.vector.tensor_copy(
                    out_sb[0:128, gi * 128 : (gi + 1) * 128], ps[0:128, 0:128]
                )
            # output in two pieces: the first (groups 0-14, ready mid-stream)
            # sits in the FIFO ring right behind the weights and flows the
            # moment the stream drains; only the small second piece pays the
            # post-last-cast generation latency.
            nc.gpsimd.dma_start(out[:, 0 : 15 * 128], out_sb[0:128, 0 : 15 * 128])
            nc.gpsimd.dma_start(out[:, 15 * 128 :], out_sb[0:128, 15 * 128 :])
    nc.compile()
    return nc


def _shard(inputs):
    x = np.asarray(inputs["x"], np.float32)
    weight = np.asarray(inputs["weight"], np.float32)[0]
    bias = np.asarray(inputs["bias"], np.float32)[0]
    xp = np.pad(x, ((0, 0), (0, 0), (1, 1), (1, 1)))  # (b, c, 50, 50)
    bias_t = bias.reshape(C_OUT, H * W).T  # (L, C_OUT)
    wflat = weight.reshape(C_IN, 3, 3, H * W, C_OUT)  # (c, kh, kw, l, o)

    in_maps = []
    for k in range(N_CORES):
        r0 = RP * k
        l0 = LP * k

        xbh = np.empty((32, 32 + XB), np.float32)
        xbh[:, 0:32] = np.eye(32, dtype=np.float32)
        xbh[:, 32:] = (
            xp[:, :, r0 : r0 + RP + 2, :].transpose(1, 2, 3, 0).reshape(32, XB)
        )

        wk = wflat[:, :, :, l0 : l0 + LP, :]  # (c, kh, kw, LP, o)
        wall = wk.transpose(2, 0, 1, 3, 4).reshape(96, 3, LP, C_OUT)
        bk = bias_t[l0 : l0 + LP, :]  # (LP, o)

        m = {"stat": xbh.astype(NPDT)}
        for t, n in enumerate(WT_SIZES):
            p0 = WT_BASE[t]
            m[f"w{t}"] = np.ascontiguousarray(
                wall[:, :, p0 : p0 + n, :].reshape(96, 3 * n * C_OUT)
            ).astype(NPDT)
            m[f"b{t}"] = np.ascontiguousarray(
                bk[p0 : p0 + n, :].reshape(1, n * C_OUT)
            ).astype(NPDT)
        in_maps.append(m)
    return in_maps


def _get_nc():
    global _nc
    if _nc is None:
        _nc = _build()
    return _nc


def _gather(results):
    # out partition 32j+8li+b holds, at col gi*128 + 32*li2 + o, the value
    # y[b, o, l] for l = 16*gi + 4*j + li  (valid only where li2 == li)
    y = np.empty((B, C_OUT, H, W), np.float32)
    li = np.arange(4)
    for k in range(N_CORES):
        arr = (
            np.asarray(results[k]["out"], np.float32)
            .reshape(4, 4, B, NGRP, 4, C_OUT)
        )  # (j, li, b, gi, li2, o)
        sel = arr[:, li, :, :, li, :]  # (li, j, b, gi, o)
        sel = sel.transpose(2, 4, 3, 1, 0)  # (b, o, gi, j, li)
        blk = sel.reshape(B, C_OUT, RP, 3, 4, 4).reshape(B, C_OUT, RP, W)
        y[:, :, RP * k : RP * (k + 1), :] = blk
    return y


def kernel(**inputs):
    nc = _get_nc()
    res = run_bass_kernel_spmd(nc, _shard(inputs), list(range(N_CORES)))
    return _gather(res.results)


# revision 31
# speedup vs baseline: 1.0434x; 1.0434x over previous
"""LocallyConnected2d (B=8, C_in=32, 48x48, C_out=32, 3x3, pad 1) on 8 trn2 cores.

Strategy: shard the spatial-location axis L = H*W across cores (6 image rows
each). Per location l the op is an (8x288)@(288x32) GEMM with location-unique
weights; weight streaming (~5.4 MB/core fp16) dominates -> memory-bound.

Device mapping per core:
  - x halo slice lives in SBUF replicated 3x with kw column shifts, laid out
    [p=(kw*32+c), (row, col, b)], so the im2col patch for any location is a
    plain strided AP slice. Partition 96 is constant 1.0 (bias row).
  - One matmul covers FOUR consecutive locations: stationary = x-view
    [97, 32] (4 locs x 8 batch), moving = W slice [97, 128] (4 locs x 32
    out-ch), out = PSUM [32, 128].  Only the block-diagonal (loc_i == loc_j)
    quarter of the output is meaningful; garbage blocks are never read.
    This cuts the instruction count 4x vs one-matmul-per-location and makes
    every moving stream 128 wide.
  - Contraction (d=288) runs as 3 kh-rounds of K=96, PSUM-accumulated, plus
    a 97th row: stationary row 96 = 1.0, moving row 96 = bias (kh=0 round)
    or 0 -> bias is folded into the weight stream, no extra instructions.
  - 4 location-quads pack onto the PE with tile_position=(0, 32j); a 16-loc
    group = 12 matmuls -> one [128, 128] PSUM tile.
  - W is host-permuted into location-range tiles ([48 x5, 32, 16] locs,
    [96, <=9216B] rows) fully contiguous in HBM, all on the gpsimd SWDGE
    queue whose descriptors spread over the 16 DMA engines ONLY when the
    partition count is a multiple of 16 (hence 96-row weight DMAs +
    separate 1-row bias DMAs + DVE memset of the ones row).  Small tail
    tiles shrink the post-stream matmul flush.  x and bias ride the two
    HWDGE queues (hardware generation, engine 0) keeping the SWDGE ring
    pure weights.
  - Output is copied PSUM->SBUF as fp16 and streamed out in 2 DMAs on the
    gpsimd queue (the first, ready mid-stream, sits in the FIFO ring right
    behind the weights); host unscrambles to NCHW and casts to fp32.
"""

import numpy as np

import concourse.bacc as bacc
import concourse.tile as tile
from concourse import mybir
from concourse.bass_utils import run_bass_kernel_spmd

B, C_IN, H, W = 8, 32, 48, 48
C_OUT = 32
N_CORES = 8
RP = H // N_CORES  # rows per core (6)
LP = RP * W  # locations per core (288)
NGRP = LP // 16  # 16-loc output groups per core (18)

DT16 = True  # fp16 operand path (halves weight traffic)
DT = mybir.dt.float16 if DT16 else mybir.dt.float32
NPDT = np.float16 if DT16 else np.float32
F32 = mybir.dt.float32

KC = 97  # contraction rows: 96 = (kw, c), row 96 = ones/bias
XF = (RP + 2) * W * B  # x3 free size (3072)
XB = (RP + 2) * 50 * B  # xbase free size: 8 halo rows x 50 padded cols (3200)
RF = W * B  # one image row of x3 free elems (384)

LG = 48  # nominal locations per weight tile
WT_SIZES = [48, 48, 48, 48, 48, 32, 16]  # small tail tiles shrink the
WT_BASE = [0, 48, 96, 144, 192, 240, 272]  # post-stream matmul flush
NT = len(WT_SIZES)
WF = 3 * LG * C_OUT  # max weight tile free size (4608 elems = 9216B rows)

_nc = None


def _tile_of(l):
    for t in range(NT - 1, -1, -1):
        if l >= WT_BASE[t]:
            return t
    raise AssertionError


def _build():
    nc = bacc.Bacc(
        "TRN2", target_bir_lowering=False, debug=False, num_devices=N_CORES
    )
    stat = nc.dram_tensor("stat", [32, 32 + XB], DT, kind="ExternalInput")
    wds = [
        nc.dram_tensor(f"w{i}", [96, 3 * n * C_OUT], DT, kind="ExternalInput")
        for i, n in enumerate(WT_SIZES)
    ]
    bds = [
        nc.dram_tensor(f"b{i}", [1, n * C_OUT], DT, kind="ExternalInput")
        for i, n in enumerate(WT_SIZES)
    ]
    out = nc.dram_tensor("out", [128, NGRP * 128], DT, kind="ExternalOutput")

    with tile.TileContext(nc) as tc:
        with (
            tc.tile_pool(name="xpool", bufs=1) as xpool,
            tc.tile_pool(name="wpool", bufs=NT) as wpool,
            tc.tile_pool(name="opool", bufs=1) as opool,
            tc.tile_pool(name="pspool", bufs=4, space="PSUM") as pspool,
            tc.tile_pool(name="repps", bufs=2, space="PSUM") as repps,
        ):
            x3 = xpool.tile([KC, XF], DT, tag="x3")
            xb = xpool.tile([32, 32 + XB], DT, tag="xb")
            out_sb = opool.tile([128, NGRP * 128], DT)

            # all bulk DMA rides the gpsimd SWDGE queue: a dma_start whose
            # partition count is a multiple of 16 spreads its descriptors
            # evenly over all 16 DMA engines; any other count pins the whole
            # transfer to ONE engine (~13 GB/s) and poisons that engine's
            # FIFO.  So weights go as [96, 9216B] transfers and the 97th
            # (bias) row rides a separate single-descriptor DMA on SP HWDGE.
            # ones row via DVE memset keeps the Pool sequencer free to start
            # generating the big stream at once.
            nc.vector.memset(x3[96:97, 0:XF], 1.0)
            # x rides HBM->SBUF once, unreplicated: [32, 8 halo rows x 50
            # padded cols x 8 batch] plus a leading 32x32 identity.  The
            # three kw-shifted x3 partition blocks are rebuilt on-chip (DVE
            # copies for kw=0; PE identity-matmul partition shifts + DVE
            # casts for kw=1,2), saving 0.4 MB of DMA-engine critical path.
            # xb leads the SWDGE ring: spread over all 16 engines it costs
            # the stream 0.85us, less than the +1.3us straggle it put on
            # engine 0 when it rode the (engine-0-pinned) HWDGE queue.
            nc.gpsimd.dma_start(xb[0:32, 0 : 32 + XB], stat[:, :])
            wts = []
            for t, n in enumerate(WT_SIZES):
                wt = wpool.tile([KC, WF], DT, tag="wt")
                nc.gpsimd.dma_start(wt[0:96, 0 : 3 * n * C_OUT], wds[t][:, :])
                nc.scalar.dma_start(wt[96:97, 0 : n * C_OUT], bds[t][:, :])
                wts.append(wt)

            for r in range(RP + 2):
                src0 = 32 + r * 50 * B
                nc.vector.tensor_copy(
                    x3[0:32, r * RF : (r + 1) * RF],
                    xb[0:32, src0 : src0 + RF],
                )
                for kw in (1, 2):
                    pr = repps.tile([128, 512], F32)
                    nc.tensor.matmul(
                        pr[32 * kw : 32 * kw + 32, 0:RF],
                        xb[0:32, 0:32],
                        xb[0:32, src0 + kw * B : src0 + kw * B + RF],
                        start=True,
                        stop=True,
                        skip_group_check=True,
                        tile_position=(0, 32 * kw),
                    )
                    nc.vector.tensor_copy(
                        x3[32 * kw : 32 * kw + 32, r * RF : (r + 1) * RF],
                        pr[32 * kw : 32 * kw + 32, 0:RF],
                    )

            for gi in range(NGRP):
                rl, qg = divmod(gi, 3)
                ps = pspool.tile([128, 512], F32)
                for kh in range(3):
                    kc = KC if kh == 0 else 96  # bias row rides kh=0 only
                    for j in range(4):
                        q0 = qg * 16 + 4 * j
                        l = rl * W + q0
                        t = _tile_of(l)
                        lgt = WT_SIZES[t]
                        ll = l - WT_BASE[t]
                        off = ((rl + kh) * W + q0) * B
                        nc.tensor.matmul(
                            ps[32 * j : 32 * j + 32, 0:128],
                            x3[0:kc, off : off + 32],
                            wts[t][0:kc, (kh * lgt + ll) * 32 : (kh * lgt + ll + 4) * 32],
                            start=(kh == 0),
                            stop=(kh == 2),
                            skip_group_check=True,
                            tile_position=(0, 32 * j),
                        )
                nc.vector.tensor_copy(
                    out_sb[0:128, gi * 128 : (gi + 1) * 128], ps[0:128, 0:128]
                )
            # output in two pieces: the first (groups 0-14, ready mid-stream)
            # sits in the FIFO ring right behind the weights and flows the
            # moment the stream drains; only the small second piece pays the
            # post-last-cast generation latency.
            nc.gpsimd.dma_start(out[:, 0 : 15 * 128], out_sb[0:128, 0 : 15 * 128])
            nc.gpsimd.dma_start(out[:, 15 * 128 :], out_sb[0:128, 15 * 128 :])
    nc.compile()
    return nc


def _shard(inputs):
    x = np.asarray(inputs["x"], np.float32)
    weight = np.asarray(inputs["weight"], np.float32)[0]
    bias = np.asarray(inputs["bias"], np.float32)[0]
    xp = np.pad(x, ((0, 0), (0, 0), (1, 1), (1, 1)))  # (b, c, 50, 50)
    bias_t = bias.reshape(C_OUT, H * W).T  # (L, C_OUT)
    wflat = weight.reshape(C_IN, 3, 3, H * W, C_OUT)  # (c, kh, kw, l, o)

    in_maps = []
    for k in range(N_CORES):
        r0 = RP * k
        l0 = LP * k

        xbh = np.empty((32, 32 + XB), np.float32)
        xbh[:, 0:32] = np.eye(32, dtype=np.float32)
        xbh[:, 32:] = (
            xp[:, :, r0 : r0 + RP + 2, :].transpose(1, 2, 3, 0).reshape(32, XB)
        )

        wk = wflat[:, :, :, l0 : l0 + LP, :]  # (c, kh, kw, LP, o)
        wall = wk.transpose(2, 0, 1, 3, 4).reshape(96, 3, LP, C_OUT)
        bk = bias_t[l0 : l0 + LP, :]  # (LP, o)

        m = {"stat": xbh.astype(NPDT)}
        for t, n in enumerate(WT_SIZES):
            p0 = WT_BASE[t]
            m[f"w{t}"] = np.ascontiguousarray(
                wall[:, :, p0 : p0 + n, :].reshape(96, 3 * n * C_OUT)
            ).astype(NPDT)
            m[f"b{t}"] = np.ascontiguousarray(
                bk[p0 : p0 + n, :].reshape(1, n * C_OUT)
            ).astype(NPDT)
        in_maps.append(m)
    return in_maps


def _get_nc():
    global _nc
    if _nc is None:
        _nc = _build()
    return _nc


def _gather(results):
    # out partition 32j+8li+b holds, at col gi*128 + 32*li2 + o, the value
    # y[b, o, l] for l = 16*gi + 4*j + li  (valid only where li2 == li)
    y = np.empty((B, C_OUT, H, W), np.float32)
    li = np.arange(4)
    for k in range(N_CORES):
        arr = (
            np.asarray(results[k]["out"], np.float32)
            .reshape(4, 4, B, NGRP, 4, C_OUT)
        )  # (j, li, b, gi, li2, o)
        sel = arr[:, li, :, :, li, :]  # (li, j, b, gi, o)
        sel = sel.transpose(2, 4, 3, 1, 0)  # (b, o, gi, j, li)
        blk = sel.reshape(B, C_OUT, RP, 3, 4, 4).reshape(B, C_OUT, RP, W)
        y[:, :, RP * k : RP * (k + 1), :] = blk
    return y


def kernel(**inputs):
    nc = _get_nc()
    res = run_bass_kernel_spmd(nc, _shard(inputs), list(range(N_CORES)))
    return _gather(res.results)


# revision 33
# speedup vs baseline: 1.0557x; 1.0118x over previous
"""LocallyConnected2d (B=8, C_in=32, 48x48, C_out=32, 3x3, pad 1) on 8 trn2 cores.

Strategy: shard the spatial-location axis L = H*W across cores (6 image rows
each). Per location l the op is an (8x288)@(288x32) GEMM with location-unique
weights; weight streaming (~5.4 MB/core fp16) dominates -> memory-bound.

Device mapping per core:
  - x halo slice lives in SBUF replicated 3x with kw column shifts, laid out
    [p=(kw*32+c), (row, col, b)], so the im2col patch for any location is a
    plain strided AP slice. Partition 96 is constant 1.0 (bias row).
  - One matmul covers FOUR consecutive locations: stationary = x-view
    [97, 32] (4 locs x 8 batch), moving = W slice [97, 128] (4 locs x 32
    out-ch), out = PSUM [32, 128].  Only the block-diagonal (loc_i == loc_j)
    quarter of the output is meaningful; garbage blocks are never read.
    This cuts the instruction count 4x vs one-matmul-per-location and makes
    every moving stream 128 wide.
  - Contraction (d=288) runs as 3 kh-rounds of K=96, PSUM-accumulated, plus
    a 97th row: stationary row 96 = 1.0, moving row 96 = bias (kh=0 round)
    or 0 -> bias is folded into the weight stream, no extra instructions.
  - 4 location-quads pack onto the PE with tile_position=(0, 32j); a 16-loc
    group = 12 matmuls -> one [128, 128] PSUM tile.
  - W is host-permuted into location-range tiles ([48 x5, 32, 16] locs,
    [96, <=9216B] rows) fully contiguous in HBM, all on the gpsimd SWDGE
    queue whose descriptors spread over the 16 DMA engines ONLY when the
    partition count is a multiple of 16 (hence 96-row weight DMAs +
    separate 1-row bias DMAs + DVE memset of the ones row).  Small tail
    tiles shrink the post-stream matmul flush.  x and bias ride the two
    HWDGE queues (hardware generation, engine 0) keeping the SWDGE ring
    pure weights.
  - Output is copied PSUM->SBUF as fp16 and streamed out in 2 DMAs on the
    gpsimd queue (the first, ready mid-stream, sits in the FIFO ring right
    behind the weights); host unscrambles to NCHW and casts to fp32.
"""

import numpy as np

import concourse.bacc as bacc
import concourse.tile as tile
from concourse import mybir
from concourse.bass_utils import run_bass_kernel_spmd

B, C_IN, H, W = 8, 32, 48, 48
C_OUT = 32
N_CORES = 8
RP = H // N_CORES  # rows per core (6)
LP = RP * W  # locations per core (288)
NGRP = LP // 16  # 16-loc output groups per core (18)

DT16 = True  # fp16 operand path (halves weight traffic)
DT = mybir.dt.float16 if DT16 else mybir.dt.float32
NPDT = np.float16 if DT16 else np.float32
F32 = mybir.dt.float32

KC = 97  # contraction rows: 96 = (kw, c), row 96 = ones/bias
XF = (RP + 2) * W * B  # x3 free size (3072)
XB = (RP + 2) * 50 * B  # xbase free size: 8 halo rows x 50 padded cols (3200)
RF = W * B  # one image row of x3 free elems (384)

LG = 48  # nominal locations per weight tile
WT_SIZES = [48, 48, 48, 48, 48, 32, 16]  # small tail tiles shrink the
WT_BASE = [0, 48, 96, 144, 192, 240, 272]  # post-stream matmul flush
NT = len(WT_SIZES)
WF = 3 * LG * C_OUT  # max weight tile free size (4608 elems = 9216B rows)

_nc = None


def _tile_of(l):
    for t in range(NT - 1, -1, -1):
        if l >= WT_BASE[t]:
            return t
    raise AssertionError


def _build():
    nc = bacc.Bacc(
        "TRN2", target_bir_lowering=False, debug=False, num_devices=N_CORES
    )
    stat = nc.dram_tensor("stat", [32, 32 + XB], DT, kind="ExternalInput")
    wds = [
        nc.dram_tensor(f"w{i}", [96, 3 * n * C_OUT], DT, kind="ExternalInput")
        for i, n in enumerate(WT_SIZES)
    ]
    bds = [
        nc.dram_tensor(f"b{i}", [1, n * C_OUT], DT, kind="ExternalInput")
        for i, n in enumerate(WT_SIZES)
    ]
    out = nc.dram_tensor("out", [128, NGRP * 128], DT, kind="ExternalOutput")

    with tile.TileContext(nc) as tc:
        with (
            tc.tile_pool(name="xpool", bufs=1) as xpool,
            tc.tile_pool(name="wpool", bufs=NT) as wpool,
            tc.tile_pool(name="opool", bufs=1) as opool,
            tc.tile_pool(name="pspool", bufs=6, space="PSUM") as pspool,
            tc.tile_pool(name="repps", bufs=2, space="PSUM") as repps,
        ):
            x3 = xpool.tile([KC, XF], DT, tag="x3")
            xb = xpool.tile([32, 32 + XB], DT, tag="xb")
            out_sb = opool.tile([128, NGRP * 128], DT)

            # all bulk DMA rides the gpsimd SWDGE queue: a dma_start whose
            # partition count is a multiple of 16 spreads its descriptors
            # evenly over all 16 DMA engines; any other count pins the whole
            # transfer to ONE engine (~13 GB/s) and poisons that engine's
            # FIFO.  So weights go as [96, 9216B] transfers and the 97th
            # (bias) row rides a separate single-descriptor DMA on SP HWDGE.
            # ones row via DVE memset keeps the Pool sequencer free to start
            # generating the big stream at once.
            nc.vector.memset(x3[96:97, 0:XF], 1.0)
            # x rides HBM->SBUF once, unreplicated: [32, 8 halo rows x 50
            # padded cols x 8 batch] plus a leading 32x32 identity.  The
            # three kw-shifted x3 partition blocks are rebuilt on-chip (DVE
            # copies for kw=0; PE identity-matmul partition shifts + DVE
            # casts for kw=1,2), saving 0.4 MB of DMA-engine critical path.
            # xb rides the SP HWDGE queue (generated in hardware, engine 0):
            # it leaves the SWDGE ring pure weights; x3 replication still
            # finishes long before the stream ends.  (Moving xb into the
            # SWDGE ring measured ~0.5us WORSE despite balancing engine 0.)
            nc.sync.dma_start(xb[0:32, 0 : 32 + XB], stat[:, :])
            wts = []
            for t, n in enumerate(WT_SIZES):
                wt = wpool.tile([KC, WF], DT, tag="wt")
                nc.gpsimd.dma_start(wt[0:96, 0 : 3 * n * C_OUT], wds[t][:, :])
                nc.scalar.dma_start(wt[96:97, 0 : n * C_OUT], bds[t][:, :])
                wts.append(wt)

            for r in range(RP + 2):
                src0 = 32 + r * 50 * B
                nc.vector.tensor_copy(
                    x3[0:32, r * RF : (r + 1) * RF],
                    xb[0:32, src0 : src0 + RF],
                )
                for kw in (1, 2):
                    pr = repps.tile([128, 512], F32)
                    nc.tensor.matmul(
                        pr[32 * kw : 32 * kw + 32, 0:RF],
                        xb[0:32, 0:32],
                        xb[0:32, src0 + kw * B : src0 + kw * B + RF],
                        start=True,
                        stop=True,
                        skip_group_check=True,
                        tile_position=(0, 32 * kw),
                    )
                    nc.vector.tensor_copy(
                        x3[32 * kw : 32 * kw + 32, r * RF : (r + 1) * RF],
                        pr[32 * kw : 32 * kw + 32, 0:RF],
                    )

            for gi in range(NGRP):
                rl, qg = divmod(gi, 3)
                ps = pspool.tile([128, 512], F32)
                for kh in range(3):
                    kc = KC if kh == 0 else 96  # bias row rides kh=0 only
                    for j in range(4):
                        q0 = qg * 16 + 4 * j
                        l = rl * W + q0
                        t = _tile_of(l)
                        lgt = WT_SIZES[t]
                        ll = l - WT_BASE[t]
                        off = ((rl + kh) * W + q0) * B
                        nc.tensor.matmul(
                            ps[32 * j : 32 * j + 32, 0:128],
                            x3[0:kc, off : off + 32],
                            wts[t][0:kc, (kh * lgt + ll) * 32 : (kh * lgt + ll + 4) * 32],
                            start=(kh == 0),
                            stop=(kh == 2),
                            skip_group_check=True,
                            tile_position=(0, 32 * j),
                        )
                nc.vector.tensor_copy(
                    out_sb[0:128, gi * 128 : (gi + 1) * 128], ps[0:128, 0:128]
                )
            # output in two pieces: the first (groups 0-14, ready mid-stream)
            # sits in the FIFO ring right behind the weights and flows the
            # moment the stream drains; only the small second piece pays the
            # post-last-cast generation latency.
            nc.gpsimd.dma_start(out[:, 0 : 15 * 128], out_sb[0:128, 0 : 15 * 128])
            nc.gpsimd.dma_start(out[:, 15 * 128 :], out_sb[0:128, 15 * 128 :])
    nc.compile()
    return nc


def _shard(inputs):
    x = np.asarray(inputs["x"], np.float32)
    weight = np.asarray(inputs["weight"], np.float32)[0]
    bias = np.asarray(inputs["bias"], np.float32)[0]
    xp = np.pad(x, ((0, 0), (0, 0), (1, 1), (1, 1)))  # (b, c, 50, 50)
    bias_t = bias.reshape(C_OUT, H * W).T  # (L, C_OUT)
    wflat = weight.reshape(C_IN, 3, 3, H * W, C_OUT)  # (c, kh, kw, l, o)

    in_maps = []
    for k in range(N_CORES):
        r0 = RP * k
        l0 = LP * k

        xbh = np.empty((32, 32 + XB), np.float32)
        xbh[:, 0:32] = np.eye(32, dtype=np.float32)
        xbh[:, 32:] = (
            xp[:, :, r0 : r0 + RP + 2, :].transpose(1, 2, 3, 0).reshape(32, XB)
        )

        wk = wflat[:, :, :, l0 : l0 + LP, :]  # (c, kh, kw, LP, o)
        wall = wk.transpose(2, 0, 1, 3, 4).reshape(96, 3, LP, C_OUT)
        bk = bias_t[l0 : l0 + LP, :]  # (LP, o)

        m = {"stat": xbh.astype(NPDT)}
        for t, n in enumerate(WT_SIZES):
            p0 = WT_BASE[t]
            m[f"w{t}"] = np.ascontiguousarray(
                wall[:, :, p0 : p0 + n, :].reshape(96, 3 * n * C_OUT)
            ).astype(NPDT)
            m[f"b{t}"] = np.ascontiguousarray(
                bk[p0 : p0 + n, :].reshape(1, n * C_OUT)
            ).astype(NPDT)
        in_maps.append(m)
    return in_maps


def _get_nc():
    global _nc
    if _nc is None:
        _nc = _build()
    return _nc


def _gather(results):
    # out partition 32j+8li+b holds, at col gi*128 + 32*li2 + o, the value
    # y[b, o, l] for l = 16*gi + 4*j + li  (valid only where li2 == li)
    y = np.empty((B, C_OUT, H, W), np.float32)
    li = np.arange(4)
    for k in range(N_CORES):
        arr = (
            np.asarray(results[k]["out"], np.float32)
            .reshape(4, 4, B, NGRP, 4, C_OUT)
        )  # (j, li, b, gi, li2, o)
        sel = arr[:, li, :, :, li, :]  # (li, j, b, gi, o)
        sel = sel.transpose(2, 4, 3, 1, 0)  # (b, o, gi, j, li)
        blk = sel.reshape(B, C_OUT, RP, 3, 4, 4).reshape(B, C_OUT, RP, W)
        y[:, :, RP * k : RP * (k + 1), :] = blk
    return y


def kernel(**inputs):
    nc = _get_nc()
    res = run_bass_kernel_spmd(nc, _shard(inputs), list(range(N_CORES)))
    return _gather(res.results)
